# revision 17
# baseline (speedup 1.0000x reference)
"""AttentionHead kernel for 8x TRN2 NeuronCores (Bass/Tile), PE-centric v3.

Reference semantics (faithful quirk: attention mixes HEADS at each position):
  q = x@Wq.T+bq ; k,v likewise ; reshape [B,S,H,Dk]
  scores[b,s,h,t] = sum_d q[b,s,h,d]*k[b,s,t,d] / sqrt(D)
  attn = softmax_t(scores) ; out[b,s,h,:] = sum_t attn*v[b,s,t,:]
  final = out@Wo.T + bo

Design: data-parallel over 16384 tokens (2048/core), weights replicated.
Per core, tokens are processed in 4 super-tiles of 512. Projections run
transposed ([out-dim-chunk, token] PSUM blocks, lhsT = W^T chunk, rhs = xT
slab). Attention runs on the PE over 8-token groups:
  - MM1: scoresT psum [128=(ut,ct,i), 128=(uh,ch,i')] via 4 quadrant matmuls
    (64-partition operands at offsets 0/64; head t=2ct+ut, h=2ch+uh), with a
    block mask (-30000 off token-diagonal) pre-accumulated via an
    identity-matmul so cross-token garbage vanishes under exp.
  - softmax: one Act exp (scale 1/sqrt(D)) -> E [128,128] bf16; the mask
    makes E block-diagonal-by-token, so column sums give exact Z.
  - MM2: EV = E^T-contraction matmul against V-rows [(ut,ct,j), d] (built by
    two PE transposes per group) + a ones-column matmul for Z.
  - normalize on DVE (batched recip + broadcast mul over 4 groups/psum bank).
  - aoT chunks for the final projection built by 2 PE transposes + 1 DVE
    scatter-copy per group; final projection contracts 8x128 chunks.
PSUM discipline: every psum tile owns a full 2KB bank - two concurrent PE
accumulation groups sharing a bank hard-crash the device (probed).
Matmul weights (lhsT) APs must have exactly ONE free dim; moving (rhs)
operands may use 2 free dims (probed) - hence group-major K/V copies but
slab-AP Q reads.
"""

import numpy as np
import ml_dtypes

import concourse.bass as bass
import concourse.mybir as mybir
from concourse import bacc
from concourse.tile import TileContext
from concourse.bass_utils import run_bass_kernel_spmd
from concourse.masks import make_identity

BF16 = ml_dtypes.bfloat16

B, S, D = 4, 4096, 1024
H, DK = 16, 64
NCORES = 8
T = B * S                 # 16384 tokens
TPC = T // NCORES         # 2048 per core
ST = 512                  # tokens per super-tile
NST = TPC // ST           # 4
NG = ST // 8              # 64 groups of 8 tokens per super-tile
MASKV = -30000.0

_CACHE = {}


def _ap(t, part_slice, free_dims, off=0):
    """AP over tile t: partition slice + explicit free dims [[stride, num],...]."""
    s = t[part_slice] if part_slice is not None else t
    return bass.AP(tensor=s.tensor, offset=s.offset + off, ap=[s.ap[0], *free_dims])


def _build_nc():
    nc = bacc.Bacc()
    dt = mybir.dt
    AF = mybir.ActivationFunctionType

    xT = nc.declare_dram_parameter("xT", [D, TPC], dt.bfloat16, isOutput=False)
    wall = nc.declare_dram_parameter("wall", [128, 4, 8, 8, 128], dt.bfloat16, isOutput=False)
    bcol = nc.declare_dram_parameter("bcol", [128, 24], dt.bfloat16, isOutput=False)
    borow = nc.declare_dram_parameter("borow", [1, D], dt.bfloat16, isOutput=False)
    maskd = nc.declare_dram_parameter("maskd", [128, 128], dt.bfloat16, isOutput=False)
    out = nc.declare_dram_parameter("out", [TPC, D], dt.float32, isOutput=True)

    inv_sqrt_d = float(1.0 / np.sqrt(np.float32(D)))

    with TileContext(nc) as tc:
        with (
            tc.tile_pool(name="wpool", bufs=1) as wpool,
            tc.tile_pool(name="xpool", bufs=2) as xpool,
            tc.tile_pool(name="qpool", bufs=2) as qpool,
            tc.tile_pool(name="kvpool", bufs=2) as kvpool,
            tc.tile_pool(name="epool", bufs=4) as epool,
            tc.tile_pool(name="aopool", bufs=2) as aopool,
            tc.tile_pool(name="fpool", bufs=2) as fpool,
            tc.tile_pool(name="pp", bufs=2, space="PSUM") as pp,
            tc.tile_pool(name="spp", bufs=3, space="PSUM") as spp,
            tc.tile_pool(name="m2p", bufs=1, space="PSUM") as m2p,
            tc.tile_pool(name="tpp", bufs=2, space="PSUM") as tpp,
        ):
            # ---- one-time loads / constants ----
            # first ST's xt + biases go first so the pipeline starts
            # while the 8MB weight wall streams in behind them.
            xts = {}

            def load_xt(st):
                t = xpool.tile([128, 8, ST], dt.bfloat16, tag="xt", name="xt")
                nc.sync.dma_start(
                    out=t,
                    in_=xT[:, st * ST : (st + 1) * ST].rearrange(
                        "(c p) s -> p c s", p=128
                    ),
                )
                xts[st] = t

            bcol_sb = wpool.tile([128, 24], dt.bfloat16)
            nc.sync.dma_start(out=bcol_sb, in_=bcol[:, :])
            load_xt(0)
            # 16 weight tiles: w_tiles[w][coh] covers out-chunk pair (2coh, 2coh+1)
            w_tiles = []
            for w in (0, 1, 2, 3):
                row = []
                for coh in range(4):
                    wt = wpool.tile([128, 2, 8, 128], dt.bfloat16, name=f"w{w}_{coh}")
                    nc.sync.dma_start(out=wt, in_=wall[:, w, 2 * coh : 2 * coh + 2])
                    row.append(wt)
                w_tiles.append(row)
            borow_sb = wpool.tile([1, D], dt.bfloat16)
            nc.sync.dma_start(out=borow_sb, in_=borow[:, :])
            mask_sb = wpool.tile([128, 128], dt.bfloat16)
            nc.sync.dma_start(out=mask_sb, in_=maskd[:, :])

            ident = wpool.tile([128, 128], dt.bfloat16)
            make_identity(nc, ident)
            swap_sb = wpool.tile([128, 128], dt.bfloat16)
            nc.vector.memset(swap_sb, 0.0)
            nc.vector.tensor_copy(swap_sb[0:64, 64:128], ident[0:64, 0:64])
            nc.vector.tensor_copy(swap_sb[64:128, 0:64], ident[64:128, 64:128])
            ones1 = wpool.tile([1, ST], dt.bfloat16)
            nc.vector.memset(ones1, 1.0)
            onescol = wpool.tile([128, 1], dt.bfloat16)
            nc.vector.memset(onescol, 1.0)
            zbias = wpool.tile([128, 1], dt.float32)
            nc.vector.memset(zbias, 0.0)

            for st in range(NST):
                tok0 = st * ST
                if st not in xts:
                    load_xt(st)
                xt = xts.pop(st)

                # ---- projections (transposed): psum chunk co = heads (2co, 2co+1)
                # qq[:, 0] = q swapped-halves, qq[:, 1] = q straight
                qq = qpool.tile([128, 2, 8, ST], dt.bfloat16, tag="qq")
                k2 = kvpool.tile([128, NG, 8, 8], dt.bfloat16, tag="k2")
                v2 = kvpool.tile([128, NG, 8, 8], dt.bfloat16, tag="v2")
                for m in range(3):
                    for co in range(8):
                        ps = pp.tile([128, ST], dt.float32, tag="pp")
                        for ci in range(8):
                            nc.tensor.matmul(
                                ps, w_tiles[m][co // 2][:, co % 2, ci, :], xt[:, ci, :],
                                start=(ci == 0), stop=(ci == 7),
                            )
                        # bias folded into the psum->SBUF copy (Act Identity)
                        if m == 0:
                            nc.scalar.activation(
                                qq[:, 1, co, :], ps, func=AF.Identity,
                                bias=bcol_sb[:, co : co + 1], scale=1.0,
                            )
                        else:
                            dst = k2 if m == 1 else v2
                            nc.scalar.activation(
                                _ap(dst, None, [[64, NG], [1, 8]], off=8 * co),
                                _ap(ps, None, [[8, NG], [1, 8]]),
                                func=AF.Identity,
                                bias=bcol_sb[:, 8 * m + co : 8 * m + co + 1],
                                scale=1.0,
                            )
                # q swapped halves (for cross-parity quadrants)
                for co in range(8):
                    ps = pp.tile([128, ST], dt.float32, tag="pp")
                    nc.tensor.matmul(ps, swap_sb, qq[:, 1, co, :], start=True, stop=True)
                    nc.vector.tensor_copy(qq[:, 0, co, :], ps)

                aoT_sb = aopool.tile([128, 8, ST], dt.bfloat16, tag="aoT")

                # ---- software-pipelined group loop: PE never waits in-order
                # on the Act-exp / DVE-copy chain of the same group.
                Es, vrs, m2ts, ao4s, fouts = {}, {}, {}, {}, {}

                def do_mm1(g):
                    # additive mask + banded scoresT matmuls
                    sp = spp.tile([128, ST], dt.float32, tag="sp")
                    nc.tensor.matmul(
                        sp[:, 0:128], ident, mask_sb,
                        start=True, stop=False, skip_group_check=True,
                    )
                    # band ut=1: cols (uh=0 <- qsw, uh=1 <- q) = qq merged
                    nc.tensor.matmul(
                        sp[64:128, 0:128], k2[64:128, g],
                        _ap(qq, slice(64, 128), [[ST, 16], [1, 8]], off=8 * g),
                        start=False, stop=True, skip_group_check=True,
                    )
                    # band ut=0: two halves (uh=0 <- q at a=1, uh=1 <- qsw at a=0)
                    nc.tensor.matmul(
                        sp[0:64, 0:64], k2[0:64, g],
                        _ap(qq, slice(0, 64), [[ST, 8], [1, 8]], off=8 * ST + 8 * g),
                        start=False, stop=True, skip_group_check=True,
                    )
                    nc.tensor.matmul(
                        sp[0:64, 64:128], k2[0:64, g],
                        _ap(qq, slice(0, 64), [[ST, 8], [1, 8]], off=8 * g),
                        start=False, stop=True, skip_group_check=True,
                    )
                    return sp

                def do_vrt(g):
                    vrp = tpp.tile([128, 1024], dt.bfloat16, tag="tp")
                    for ut in range(2):
                        o = 64 * ut
                        nc.tensor.transpose(
                            vrp[o : o + 64, 0:64], v2[o : o + 64, g],
                            ident[o : o + 64, o : o + 64],
                        )
                    vr_sb = epool.tile([128, 64], dt.bfloat16, tag="vr_sb")
                    nc.vector.tensor_copy(vr_sb, vrp[:, 0:64])
                    vrs[g] = vr_sb

                def do_exp(g, sp):
                    E_sb = epool.tile([128, 128], dt.bfloat16, tag="E")
                    nc.scalar.activation(
                        E_sb, sp[:, 0:128], func=AF.Exp,
                        bias=zbias[:, 0:1], scale=inv_sqrt_d,
                    )
                    Es[g] = E_sb

                def do_mm2(g):
                    bidx = g % 4
                    if bidx == 0:
                        m2ts[g // 4] = m2p.tile([128, ST], dt.float32, tag="m2", name="m2t")
                    m2t = m2ts[g // 4]
                    c0 = 65 * bidx
                    nc.tensor.matmul(m2t[:, c0 : c0 + 64], Es[g], vrs[g], start=True, stop=True)
                    nc.tensor.matmul(m2t[:, c0 + 64 : c0 + 65], Es[g], onescol, start=True, stop=True)
                    del Es[g], vrs[g]

                def do_norm(b):
                    m2t = m2ts[b]
                    rz = epool.tile([128, 4], dt.float32, tag="rz")
                    nc.vector.reciprocal(rz, _ap(m2t, None, [[65, 4]], off=64))
                    ao4 = epool.tile([128, 4, 64], dt.bfloat16, tag="ao4")
                    nc.vector.tensor_mul(
                        ao4,
                        _ap(m2t, None, [[65, 4], [1, 64]]),
                        _ap(rz, None, [[1, 4], [0, 64]]),
                    )
                    ao4s[b] = ao4
                    del m2ts[b]

                def do_aot(b):
                    ao4 = ao4s.pop(b)
                    aop = tpp.tile([128, 1024], dt.bfloat16, tag="tp")
                    for b2 in range(4):
                        for u in range(2):
                            o = 64 * u
                            nc.tensor.transpose(
                                aop[o : o + 64, 64 * b2 : 64 * b2 + 64],
                                ao4[o : o + 64, b2, :],
                                ident[o : o + 64, o : o + 64],
                            )
                    for b2 in range(4):
                        nc.vector.tensor_copy(
                            _ap(aoT_sb, None, [[ST, 8], [1, 8]], off=8 * (4 * b + b2)),
                            _ap(aop, None, [[8, 8], [1, 8]], off=64 * b2),
                        )

                def do_final(tb):
                    # one 128-token block; DMA per 256-token pair
                    jb, j = tb // 2, tb % 2
                    if j == 0:
                        fouts[jb] = fpool.tile([128, 2, 2, ST], dt.float32, tag="fout", name="fout")
                    fout = fouts[jb]
                    for half in range(2):
                        fp = pp.tile([128, ST], dt.float32, tag="pp")
                        nc.tensor.matmul(
                            fp, ones1[:, 0:128],
                            borow_sb[:, ST * half : ST * half + ST],
                            start=True, stop=False,
                        )
                        for ch in range(8):
                            for cp in range(2):
                                nc.tensor.matmul(
                                    fp[:, 256 * cp : 256 * cp + 256],
                                    aoT_sb[:, ch, 128 * tb : 128 * tb + 128],
                                    w_tiles[3][2 * half + cp][:, :, ch, :],
                                    start=False, stop=(ch == 7),
                                    skip_group_check=True,
                                )
                        nc.scalar.activation(fout[:, j, half, :], fp, func=AF.Copy)
                    if j == 1:
                        gtok = tok0 + 256 * jb
                        nc.sync.dma_start(
                            out=out[gtok : gtok + 256, :].rearrange(
                                "(j p) o -> p j o", p=128
                            ),
                            in_=fouts.pop(jb).rearrange("p j h s -> p j (h s)"),
                        )

                LAG_MM2, LAG_AOT, LAG_FIN = 3, 9, 25
                for g in range(NG + LAG_FIN + 16):
                    if g < NG:
                        sp = do_mm1(g)
                        do_vrt(g)
                        do_exp(g, sp)
                    gm = g - LAG_MM2
                    if 0 <= gm < NG:
                        do_mm2(gm)
                        if gm % 4 == 3:
                            do_norm(gm // 4)
                    ga = g - LAG_AOT
                    if ga >= 0 and ga % 4 == 0 and ga // 4 < NG // 4:
                        do_aot(ga // 4)
                    gf = g - LAG_FIN
                    if gf >= 0 and gf % 16 == 0 and gf // 16 < 4:
                        do_final(gf // 16)

    nc.compile()
    return nc


def _host_prep(x, Wq, bq, Wk, bk, Wv, bv, Wo, bo):
    xt = np.ascontiguousarray(x.reshape(T, D).T).astype(BF16)  # [D, T]
    # wall[p, w, ci, co, :] = W^T[128*ci + p, 128*co : 128*co + 128]
    wall = np.empty((128, 4, 8, 8, 128), dtype=BF16)
    for w, W in enumerate((Wq, Wk, Wv, Wo)):
        WT = W.T  # [in, out]
        wall[:, w] = WT.reshape(8, 128, 8, 128).transpose(1, 2, 0, 3).astype(BF16)
    bcol = np.stack([b.reshape(8, 128).T for b in (bq, bk, bv)], axis=1)
    bcol = bcol.reshape(128, 24).astype(BF16)  # [p, 8m+co] -> b_m[128co+p]
    borow = bo.astype(BF16)[None, :]
    ii = np.arange(128) % 8
    mask = np.where(ii[:, None] == ii[None, :], 0.0, MASKV).astype(BF16)
    return xt, wall, bcol, borow, mask


def kernel(x, Wq, bq, Wk, bk, Wv, bv, Wo, bo, _trace=False):
    x = np.asarray(x, dtype=np.float32)
    arrs = [np.asarray(a, dtype=np.float32) for a in (Wq, bq, Wk, bk, Wv, bv, Wo, bo)]
    xt, wall, bcol, borow, mask = _host_prep(x, *arrs)

    if "nc" not in _CACHE:
        _CACHE["nc"] = _build_nc()
    nc = _CACHE["nc"]

    in_maps = []
    for c in range(NCORES):
        in_maps.append(
            {
                "wall": wall,
                "bcol": bcol,
                "borow": borow,
                "maskd": mask,
                "xT": np.ascontiguousarray(xt[:, c * TPC : (c + 1) * TPC]),
            }
        )

    _CACHE["in_maps"] = in_maps
    res = run_bass_kernel_spmd(nc, in_maps, core_ids=list(range(NCORES)), trace=_trace)
    _CACHE["last_result"] = res
    out = np.concatenate([res.results[c]["out"] for c in range(NCORES)], axis=0)
    return out.reshape(B, S, D)
